# revision 1
# baseline (speedup 1.0000x reference)
"""Local (causal) attention block on 8 TRN2 NeuronCores.

Reference computation (B=2, T=2048, C=1024, H=16, D=64):
    q,k,v = x@Wq.T, x@Wk.T, x@Wv.T          (per-head D=64)
    att   = softmax(causal_mask(q k^T / sqrt(D)))
    out   = (att v) @ Wo.T
(The reference's "window" band mask reduces exactly to the plain strict
causal mask, so this is full causal attention.)

Sharding (SPMD-uniform across the 8 cores):
  core c: batch b = c//4, head-group g = c%4 (heads 4g..4g+3),
  output-channel group g (columns 256g..256g+255).
  - QKV projections head-sharded: each core computes q^T,K^T,V for its 4
    heads, all 2048 positions (f32r matmuls on f32 inputs).
  - Attention: full causal for its 4 heads (S^T layout: kv on partitions,
    q on free axis; exp on ScalarE; rowsum via a ones-column appended to V;
    normalization via gpsimd partition-broadcast of 1/rowsum).
  - O^T (bf16) exchanged between the 4 cores of a batch with an AllGather.
  - Output projection oc-sharded: each core computes out[:, 256g:256g+256]
    for the whole batch (bf16 matmuls).
Host side only shards inputs and concatenates the per-core output slices.
"""

import sys

for _p in ("/opt/trn_rl_repo",):
    if _p not in sys.path:
        sys.path.append(_p)

import numpy as np

import concourse.bass as bass
import concourse.mybir as mybir
import concourse.tile as tile
from concourse import bacc
from concourse.bass import ts
from concourse.bass_utils import run_bass_kernel_spmd

B, T, C = 2, 2048, 1024
H, D = 16, 64
SCALE = 1.0 / np.sqrt(D)
N_CORES = 8
HPC = H // 4          # heads per core = 4
COC = C // 4          # channels per core = 256
F32 = mybir.dt.float32
BF16 = mybir.dt.bfloat16
F32R = mybir.dt.float32r
NEG = -1.0e5          # additive mask value (pre-scale)


def r(ap):
    """view an f32 AP as f32r for full-rate fp32 matmul"""
    return ap.bitcast(F32R)


def attention_qchunk(nc, psum, work, qT_sb, kT_sb, v_sb, masks, otall_c, qc):
    """Causal attention for one 512-wide q-chunk, all 4 local heads.

    S^T layout (kv on partitions, q free). Head pairs are row-packed on
    the PE (rows 0-63 / 64-127 via tile_position). Softmax normalization
    is batched per chunk: the V ones-column puts each head's rowsum in
    psum row D; rowsums are collected into a [4, 512] tile, reciprocated
    in one DVE op, partition-broadcast, and multiplied into O^T.
    """
    nkv = 4 * (qc + 1)
    # rowsums parked on partitions {0,32,64,96} (the only legal AP start
    # partitions); memset so the batched reciprocal sees no garbage
    rsums = work.tile([128, 512], F32, tag="rsums")
    nc.vector.memset(rsums[:], 1.0)
    ot_ps = {}
    for pair in range(HPC // 2):
        h0, h1 = 2 * pair, 2 * pair + 1
        for i in (0, 1):
            ot_ps[2 * pair + i] = psum.tile(
                [D + 1, 512], F32, tag=f"o{i}", name=f"o{i}"
            )
        for k in range(nkv):
            s_ps = [
                psum.tile([128, 512], F32, tag=f"s{i}", name=f"s{i}")
                for i in (0, 1)
            ]
            for i, h in ((0, h0), (1, h1)):
                bp = 64 * (h % 2)
                nc.tensor.matmul(
                    s_ps[i][:],
                    kT_sb[bp : bp + 64, h // 2, ts(k, 128)],
                    qT_sb[bp : bp + 64, h // 2, ts(qc, 512)],
                    start=True,
                    stop=True,
                    tile_position=(bp, 0),
                )
            m = k - 4 * qc
            for i, h in ((0, h0), (1, h1)):
                pt = work.tile([128, 512], BF16, tag="pt")
                nc.scalar.activation(
                    pt[:],
                    s_ps[i][:],
                    mybir.ActivationFunctionType.Exp,
                    scale=float(SCALE),
                )
                if m >= 0:  # diagonal tile: zero the disallowed region
                    nc.vector.tensor_mul(pt[:], pt[:], masks[:, m, :])
                nc.tensor.matmul(
                    ot_ps[h][:],
                    v_sb[:, k, h, :],
                    pt[:],
                    start=(k == 0),
                    stop=(k == nkv - 1),
                )
        for i, h in ((0, h0), (1, h1)):
            nc.vector.tensor_copy(
                rsums[32 * h : 32 * h + 1, :], ot_ps[h][D : D + 1, :]
            )
    nc.vector.reciprocal(rsums[:], rsums[:])
    for h in range(HPC):
        # stage this head's reciprocals at partition 0: partition_broadcast
        # reads partition 0 of its source on hardware
        stg = work.tile([1, 512], F32, tag="stg")
        nc.vector.tensor_copy(stg[:], rsums[32 * h : 32 * h + 1, :])
        bcast = work.tile([64, 512], F32, tag="bcast")
        nc.gpsimd.partition_broadcast(bcast[:], stg[:])
        nc.vector.tensor_mul(
            otall_c[64 * (h % 2) : 64 * (h % 2) + 64, h // 2, :],
            ot_ps[h][0:D, :],
            bcast[:],
        )


def build_nc():
    nc = bacc.Bacc(
        "TRN2",
        target_bir_lowering=False,
        debug=False,
        num_devices=N_CORES,
    )
    xT_d = nc.dram_tensor("xT", [C, T], F32, kind="ExternalInput").ap()
    wqT_d = nc.dram_tensor("wqT", [C, COC], F32, kind="ExternalInput").ap()
    wkT_d = nc.dram_tensor("wkT", [C, COC], F32, kind="ExternalInput").ap()
    wvT_d = nc.dram_tensor("wvT", [C, COC], F32, kind="ExternalInput").ap()
    woT_d = nc.dram_tensor("woT", [C, COC], F32, kind="ExternalInput").ap()
    out_d = nc.dram_tensor("out", [T, COC], F32, kind="ExternalOutput").ap()

    NQC = T // 512     # 4 q-chunks of 512
    NKT = T // 128     # 16 kv tiles of 128
    NCT = C // 128     # 8 contraction tiles

    with tile.TileContext(nc) as tc:
        with (
            tc.tile_pool(name="main", bufs=1) as main,
            tc.tile_pool(name="work", bufs=4) as work,
            tc.tile_pool(name="dram", bufs=2, space="DRAM") as dram,
        ):
            # ---- long-lived SBUF tensors ----
            qT_sb = main.tile([128, 2, T], BF16)         # [co 256, t]
            kT_sb = main.tile([128, 2, T], BF16)
            v_sb = main.tile([128, NKT, HPC, D + 1], BF16)  # V + ones col
            # own normalized O^T / gathered O^T, one tile per q-chunk so the
            # exchange + output projection pipeline per chunk
            otall_c = [
                main.tile([128, 2, 512], BF16, name=f"otall{j}") for j in range(NQC)
            ]
            otfull_c = [
                main.tile([128, NCT, 512], BF16, name=f"otfull{j}")
                for j in range(NQC)
            ]
            woT_bf = main.tile([128, NCT, COC], BF16)
            masks = main.tile([128, 4, 512], BF16)       # 1/0 multiplicative

            # ---- phase 1: projections (f32r), x^T streamed in t-chunks ----
            xT_r = xT_d.rearrange("(a p) t -> p a t", p=128)
            with (
                tc.tile_pool(name="p1w", bufs=1) as p1w,
                tc.tile_pool(name="p1ws", bufs=2) as p1ws,
                tc.tile_pool(name="p1x", bufs=2) as p1x,
                tc.tile_pool(name="p1psum", bufs=2, space="PSUM") as psum_p1,
            ):
                wq_sb = p1w.tile([128, NCT, COC], BF16)
                wk_sb = p1w.tile([128, NCT, COC], BF16)
                wv_sb = p1w.tile([128, NCT, COC], BF16)
                for w_sb, w_d in ((wq_sb, wqT_d), (wk_sb, wkT_d), (wv_sb, wvT_d)):
                    wst = p1ws.tile([128, NCT, COC], F32, tag="wst")
                    nc.sync.dma_start(
                        out=wst[:], in_=w_d.rearrange("(a p) t -> p a t", p=128)
                    )
                    nc.vector.tensor_copy(w_sb[:], wst[:])

                for tj in range(NQC):
                    xch = p1x.tile([128, NCT, 512], F32, tag="xch")
                    nc.sync.dma_start(out=xch[:], in_=xT_r[:, :, ts(tj, 512)])
                    xbf = p1x.tile([128, NCT, 512], BF16, tag="xbf")
                    nc.vector.tensor_copy(xbf[:], xch[:])

                    # q^T and K^T: [co, t] = sum_c W[c, co]^T x^T[c, t]
                    for w_sb, dst in ((wq_sb, qT_sb), (wk_sb, kT_sb)):
                        for co in range(2):
                            ps = psum_p1.tile([128, 512], F32, tag="psA")
                            for ci in range(NCT):
                                nc.tensor.matmul(
                                    ps[:],
                                    w_sb[:, ci, ts(co, 128)],
                                    xbf[:, ci, :],
                                    start=(ci == 0),
                                    stop=(ci == NCT - 1),
                                )
                            nc.vector.tensor_copy(dst[:, co, ts(tj, 512)], ps[:])

                    # V: [t, co] = sum_c x^T[c, t]^T W_v^T[c, co]; aug layout
                    for tl in range(4):
                        tt = 4 * tj + tl
                        ps = psum_p1.tile([128, COC], F32, tag="psB")
                        for ci in range(NCT):
                            nc.tensor.matmul(
                                ps[:],
                                xbf[:, ci, ts(tl, 128)],
                                wv_sb[:, ci, :],
                                start=(ci == 0),
                                stop=(ci == NCT - 1),
                            )
                        nc.vector.tensor_copy(
                            v_sb[:, tt, :, 0:D],
                            ps[:].rearrange("p (h d) -> p h d", h=HPC),
                        )
                nc.vector.memset(v_sb[:, :, :, D], 1.0)

            # weights for the output projection (needed from chunk 0)
            wo_f32 = main.tile([128, NCT, COC], F32)
            nc.sync.dma_start(
                out=wo_f32[:], in_=woT_d.rearrange("(a p) t -> p a t", p=128)
            )
            for ci in range(NCT):
                nc.vector.tensor_copy(woT_bf[:, ci, :], wo_f32[:, ci, :])

            # ---- phase 2-4 pipeline per q-chunk:
            #      attention -> AllGather(O^T chunk) -> out projection ----
            mk32 = main.tile([128, 4, 512], F32)
            for m in range(4):
                nc.gpsimd.memset(mk32[:, m, :], 1.0)
                nc.gpsimd.affine_select(
                    out=mk32[:, m, :],
                    in_=mk32[:, m, :],
                    pattern=[[1, 512]],
                    compare_op=mybir.AluOpType.is_ge,
                    fill=0.0,
                    base=-128 * m,
                    channel_multiplier=-1,
                )
                nc.vector.tensor_copy(masks[:, m, :], mk32[:, m, :])

            with tc.tile_pool(name="psum2", bufs=2, space="PSUM") as psum:

                def out_proj_chunk(qc):
                    # oc-sharded output projection for one 512-row q-chunk
                    for ql in range(4):
                        ps = psum.tile(
                            [128, COC], F32, tag=f"s{ql % 2}", name="po"
                        )
                        for ci in range(NCT):
                            nc.tensor.matmul(
                                ps[:],
                                otfull_c[qc][:, ci, ts(ql, 128)],
                                woT_bf[:, ci, :],
                                start=(ci == 0),
                                stop=(ci == NCT - 1),
                            )
                        ot = work.tile([128, COC], F32, tag="outst")
                        nc.vector.tensor_copy(ot[:], ps[:])
                        nc.sync.dma_start(
                            out=out_d[ts(4 * qc + ql, 128), :], in_=ot[:]
                        )

                for qc in range(NQC):
                    attention_qchunk(
                        nc, psum, work, qT_sb, kT_sb, v_sb, masks, otall_c[qc], qc
                    )

                    # exchange this chunk's O^T across the batch's 4 cores
                    bounce_in = dram.tile(
                        [COC, 512], BF16, tag="bin", name=f"bin{qc}"
                    )
                    bounce_out = dram.tile(
                        [C, 512], BF16, tag="bout", name=f"bout{qc}"
                    )
                    for i in range(2):
                        nc.sync.dma_start(
                            out=bounce_in[ts(i, 128), :], in_=otall_c[qc][:, i, :]
                        )
                    nc.gpsimd.collective_compute(
                        "AllGather",
                        mybir.AluOpType.bypass,
                        replica_groups=[[0, 1, 2, 3], [4, 5, 6, 7]],
                        ins=[bounce_in.opt()],
                        outs=[bounce_out.opt()],
                    )
                    nc.sync.dma_start(
                        out=otfull_c[qc][:],
                        in_=bounce_out[:].rearrange("(a p) t -> p a t", p=128),
                    )
                    # emit the PREVIOUS chunk's projection here so its psum
                    # slot reuse never makes the next chunk's attention wait
                    # on this chunk's AllGather
                    if qc > 0:
                        out_proj_chunk(qc - 1)
                out_proj_chunk(NQC - 1)

    nc.compile()
    return nc


_NC_CACHE = None


def _get_nc():
    global _NC_CACHE
    if _NC_CACHE is None:
        _NC_CACHE = build_nc()
    return _NC_CACHE


def make_in_maps(x, Wq, Wk, Wv, Wo):
    x = np.asarray(x, dtype=np.float32)
    in_maps = []
    for c in range(N_CORES):
        b, g = c // 4, c % 4
        sl = slice(COC * g, COC * g + COC)
        in_maps.append(
            {
                "xT": np.ascontiguousarray(x[b].T),
                "wqT": np.ascontiguousarray(np.asarray(Wq)[sl, :].T),
                "wkT": np.ascontiguousarray(np.asarray(Wk)[sl, :].T),
                "wvT": np.ascontiguousarray(np.asarray(Wv)[sl, :].T),
                "woT": np.ascontiguousarray(np.asarray(Wo)[sl, :].T),
            }
        )
    return in_maps


def assemble(results):
    out = np.empty((B, T, C), dtype=np.float32)
    for c in range(N_CORES):
        b, g = c // 4, c % 4
        out[b, :, COC * g : COC * g + COC] = results[c]["out"]
    return out


def kernel(x, Wq, Wk, Wv, Wo):
    nc = _get_nc()
    in_maps = make_in_maps(x, Wq, Wk, Wv, Wo)
    res = run_bass_kernel_spmd(nc, in_maps, list(range(N_CORES)))
    return assemble(res.results)


if __name__ == "__main__":
    rng = np.random.default_rng(0)
    x = rng.standard_normal((B, T, C), dtype=np.float32)
    s = 1.0 / np.sqrt(C)
    ws = [
        rng.uniform(-s, s, size=(C, C)).astype(np.float32) for _ in range(4)
    ]
    out = kernel(x, *ws)
    print("kernel ran; out", out.shape, out.dtype)



# revision 4
# speedup vs baseline: 1.1511x; 1.1511x over previous
"""Local (causal) attention block on 8 TRN2 NeuronCores.

Reference computation (B=2, T=2048, C=1024, H=16, D=64):
    q,k,v = x@Wq.T, x@Wk.T, x@Wv.T          (per-head D=64)
    att   = softmax(causal_mask(q k^T / sqrt(D)))
    out   = (att v) @ Wo.T
(The reference's "window" band mask reduces exactly to the plain strict
causal mask, so this is full causal attention.)

Sharding (SPMD-uniform across the 8 cores):
  core c: batch b = c//4, head-group g = c%4 (heads 4g..4g+3),
  output-channel group g (columns 256g..256g+255).

Per-core pipeline (per 256-wide q chunk, software-pipelined):
  - QKV projections in f32r (full-rate fp32) -> no input casts.
  - Attention: scores for all 4 heads of a kv tile land in one
    [128,1024] psum tile (head h at free offset 256*(2*(h%2)+h//2), so
    row-packed head pairs drain to different banks); ONE batched exp per
    kv tile on ScalarE; causal diag truncated to the valid q range with a
    single reusable [128,128] triangular mask; AV with a ones-column on V
    for rowsums; both heads of a pair accumulate into ONE psum bank
    (per-element has_written handles region-wise start).
  - Normalization: rowsum rows -> reciprocal_approx_fast -> gpsimd
    partition_broadcast -> DVE muls into bf16 O^T.
  - O^T chunk AllGathered across the batch's 4 cores (bf16, DRAM bounce).
  - Output projection transposed (stationary Wo slices, moving O^T,
    N=256) -> out^T [COC, T]; host transposes back.
"""

import sys

for _p in ("/opt/trn_rl_repo",):
    if _p not in sys.path:
        sys.path.append(_p)

import numpy as np

import concourse.bass as bass
import concourse.mybir as mybir
import concourse.tile as tile
from concourse import bacc
from concourse.bass import ts
from concourse.bass_utils import run_bass_kernel_spmd

B, T, C = 2, 2048, 1024
H, D = 16, 64
SCALE = 1.0 / np.sqrt(D)
N_CORES = 8
HPC = H // 4          # heads per core = 4
COC = C // 4          # channels per core = 256
F32 = mybir.dt.float32
BF16 = mybir.dt.bfloat16
F32R = mybir.dt.float32r

NQC = T // 256        # 8 q-chunks of 256
NKT = T // 128        # 16 kv tiles of 128
NCT = C // 128        # 8 contraction tiles
NTB = T // 512        # 4 projection t-blocks


def r(ap):
    """view an f32 AP as f32r for full-rate fp32 matmul"""
    return ap.bitcast(F32R)


def off(h):
    # free offset of head h inside the [128,1024] score tile; row-packed
    # pairs (h, h+1) land in different psum banks
    return 256 * (2 * (h % 2) + h // 2)


def build_nc():
    nc = bacc.Bacc(
        "TRN2",
        target_bir_lowering=False,
        debug=False,
        num_devices=N_CORES,
    )
    xT_d = nc.dram_tensor("xT", [C, T], F32, kind="ExternalInput").ap()
    wqT_d = nc.dram_tensor("wqT", [C, COC], F32, kind="ExternalInput").ap()
    wkT_d = nc.dram_tensor("wkT", [C, COC], F32, kind="ExternalInput").ap()
    wvT_d = nc.dram_tensor("wvT", [C, COC], F32, kind="ExternalInput").ap()
    woT_d = nc.dram_tensor("woT", [C, COC], F32, kind="ExternalInput").ap()
    # transposed output: out^T [COC, T]; host transposes back
    out_d = nc.dram_tensor("out", [COC, T], F32, kind="ExternalOutput").ap()

    xT_r = xT_d.rearrange("(a p) t -> p a t", p=128)
    outT_r = out_d.rearrange("(g p) t -> p g t", p=128)

    with tile.TileContext(nc) as tc:
        with (
            tc.tile_pool(name="main", bufs=1) as main,
            tc.tile_pool(name="work", bufs=4) as work,
            tc.tile_pool(name="work2", bufs=2) as work2,
            tc.tile_pool(name="exf", bufs=3) as exf,
            tc.tile_pool(name="p1x", bufs=2) as p1x,
            tc.tile_pool(name="psA", bufs=2, space="PSUM") as psA,
            tc.tile_pool(name="psB", bufs=1, space="PSUM") as psB,
            tc.tile_pool(name="psC", bufs=2, space="PSUM") as psC,
            tc.tile_pool(name="dram", bufs=2, space="DRAM") as dram,
        ):
            # ---- long-lived SBUF tensors ----
            qT_sb = main.tile([128, 2, T], BF16)   # [64*(h%2)+c, pair, t]
            kT_sb = main.tile([128, 2, T], BF16)
            v_sb = main.tile([128, NKT, HPC, D + 1], BF16)  # V + ones col
            woT_bf = main.tile([128, NCT, COC], BF16)
            tri = main.tile([128, 128], BF16)      # lower-tri 1/0 mask

            # ---- weights (f32, used directly via f32r matmuls) ----
            wq_sb = main.tile([128, NCT, COC], F32R)
            wk_sb = main.tile([128, NCT, COC], F32R)
            wv_sb = main.tile([128, NCT, COC], F32R)
            nc.sync.dma_start(
                out=wq_sb[:],
                in_=wqT_d.rearrange("(a p) t -> p a t", p=128).bitcast(F32R),
            )
            # x chunk 0 early so projections can start ASAP
            xts = {}
            xts[0] = p1x.tile([128, NCT, 512], F32R, tag="xch", name="xch0")
            nc.sync.dma_start(
                out=xts[0][:], in_=xT_r[:, :, ts(0, 512)].bitcast(F32R)
            )
            nc.sync.dma_start(
                out=wk_sb[:],
                in_=wkT_d.rearrange("(a p) t -> p a t", p=128).bitcast(F32R),
            )
            nc.sync.dma_start(
                out=wv_sb[:],
                in_=wvT_d.rearrange("(a p) t -> p a t", p=128).bitcast(F32R),
            )

            # triangular mask: keep (1.0) where q >= kv within the block
            mk32 = work2.tile([128, 128], F32, tag="mk32")
            nc.gpsimd.memset(mk32[:], 1.0)
            nc.gpsimd.affine_select(
                out=mk32[:],
                in_=mk32[:],
                pattern=[[1, 128]],
                compare_op=mybir.AluOpType.is_ge,
                fill=0.0,
                base=0,
                channel_multiplier=-1,
            )
            nc.vector.tensor_copy(tri[:], mk32[:])
            # ones column for rowsums
            nc.vector.memset(v_sb[:, :, :, D], 1.0)

            def proj(t, xch):
                # q^T, K^T: [co, t] = sum_c W[c, co]^T x^T[c, t]  (f32r)
                for w_sb, dst in ((wq_sb, qT_sb), (wk_sb, kT_sb)):
                    for co in range(2):
                        ps = psC.tile([128, 512], F32, tag="pp")
                        for ci in range(NCT):
                            nc.tensor.matmul(
                                ps[:],
                                w_sb[:, ci, ts(co, 128)],
                                xch[:, ci, :],
                                start=(ci == 0),
                                stop=(ci == NCT - 1),
                            )
                        nc.vector.tensor_copy(dst[:, co, ts(t, 512)], ps[:])
                # V: [t, (h d)] = sum_c x^T[c, t]^T W_v^T[c, co]
                for tl in range(4):
                    tt = 4 * t + tl
                    ps = psC.tile([128, 512], F32, tag="pp")
                    for ci in range(NCT):
                        nc.tensor.matmul(
                            ps[:, 0:COC],
                            xch[:, ci, ts(tl, 128)],
                            wv_sb[:, ci, :],
                            start=(ci == 0),
                            stop=(ci == NCT - 1),
                        )
                    nc.vector.tensor_copy(
                        v_sb[:, tt, :, 0:D],
                        ps[:, 0:COC].rearrange("p (h d) -> p h d", h=HPC),
                    )

            def attn(qc):
                """Causal attention for one 256-wide q-chunk, 4 heads.

                Returns the gathered O^T tile for this chunk."""
                nk = 2 * qc + 2
                ot = [
                    psB.tile([D + 1, 512], F32, tag=f"ot{p}", name=f"ot{p}_{qc}")
                    for p in range(2)
                ]
                started = [False, False]
                for k in range(nk):
                    m = k - 2 * qc  # >=0: diagonal tiles
                    qlo = 128 if m == 1 else 0
                    sc = psA.tile([128, 1024], F32, tag="sc")
                    for h in range(HPC):
                        p, j = h // 2, h % 2
                        o = off(h)
                        nc.tensor.matmul(
                            sc[:, o + qlo : o + 256],
                            kT_sb[64 * j : 64 * j + 64, p, ts(k, 128)],
                            qT_sb[64 * j : 64 * j + 64, p,
                                  256 * qc + qlo : 256 * qc + 256],
                            start=True,
                            stop=True,
                            tile_position=(64 * j, 0),
                        )
                    pt = work.tile([128, 1024], BF16, tag="pt")
                    if m == 1:
                        # only the upper q-half is valid on the last diag tile
                        sc_h = sc[:].rearrange("p (g q) -> p g q", g=4)[:, :, 128:256]
                        pt_h = pt[:].rearrange("p (g q) -> p g q", g=4)[:, :, 128:256]
                        nc.scalar.activation(
                            pt_h, sc_h,
                            mybir.ActivationFunctionType.Exp,
                            scale=float(SCALE),
                        )
                    else:
                        nc.scalar.activation(
                            pt[:], sc[:],
                            mybir.ActivationFunctionType.Exp,
                            scale=float(SCALE),
                        )
                    if m >= 0:  # triangular region at q offset 128*m per head
                        for h in range(HPC):
                            reg = off(h) + 128 * m
                            nc.vector.tensor_mul(
                                pt[:, reg : reg + 128],
                                pt[:, reg : reg + 128],
                                tri[:],
                            )
                    for h in range(HPC):
                        p, j = h // 2, h % 2
                        pos = 256 * j
                        nc.tensor.matmul(
                            ot[p][:, pos + qlo : pos + 256],
                            v_sb[:, k, h, :],
                            pt[:, off(h) + qlo : off(h) + 256],
                            start=(not started[p]),
                            stop=(k == nk - 1 and j == 1),
                        )
                        started[p] = True

                # ---- normalization ----
                rs = work2.tile([128, 512], F32, tag="rs")
                nc.gpsimd.memset(rs[:], 1.0)
                for p in range(2):
                    nc.vector.tensor_copy(
                        rs[32 * p : 32 * p + 1, :], ot[p][D : D + 1, :]
                    )
                nc.vector.reciprocal_approx_fast(rs[:], rs[:])
                otall = exf.tile([128, 2, 256], BF16, tag="otall",
                                 name=f"otall{qc}")
                for p in range(2):
                    stg = work2.tile([1, 512], F32, tag=f"stg{p}")
                    nc.gpsimd.tensor_copy(stg[:], rs[32 * p : 32 * p + 1, :])
                    bc = work2.tile([64, 512], F32, tag=f"bc{p}")
                    nc.gpsimd.partition_broadcast(bc[:], stg[:])
                    for j in range(2):
                        nc.vector.tensor_mul(
                            otall[64 * j : 64 * j + 64, p, :],
                            ot[p][0:D, ts(j, 256)],
                            bc[:, ts(j, 256)],
                        )

                # ---- exchange across the batch's 4 cores ----
                bin_ = dram.tile([COC, 256], BF16, tag="bin", name=f"bin{qc}")
                bout = dram.tile([C, 256], BF16, tag="bout", name=f"bout{qc}")
                for a in range(2):
                    nc.sync.dma_start(
                        out=bin_[ts(a, 128), :], in_=otall[:, a, :]
                    )
                nc.gpsimd.collective_compute(
                    "AllGather",
                    mybir.AluOpType.bypass,
                    replica_groups=[[0, 1, 2, 3], [4, 5, 6, 7]],
                    ins=[bin_.opt()],
                    outs=[bout.opt()],
                )
                otfull = exf.tile([128, NCT, 256], BF16, tag="otfull",
                                  name=f"otfull{qc}")
                nc.sync.dma_start(
                    out=otfull[:],
                    in_=bout[:].rearrange("(a p) t -> p a t", p=128),
                )
                return otfull

            def outproj(qc, otfull):
                # out^T[co, q] = sum_c Wo^T[c, co]^T O^T[c, q]  (bf16)
                po = psC.tile([128, 512], F32, tag="pp")
                for g in range(2):
                    for ci in range(NCT):
                        nc.tensor.matmul(
                            po[:, ts(g, 256)],
                            woT_bf[:, ci, ts(g, 128)],
                            otfull[:, ci, :],
                            start=(ci == 0),
                            stop=(ci == NCT - 1),
                        )
                osb = work.tile([128, 512], F32, tag="outst")
                nc.vector.tensor_copy(osb[:], po[:])
                nc.sync.dma_start(
                    out=outT_r[:, :, ts(qc, 256)],
                    in_=osb[:].rearrange("p (g q) -> p g q", g=2),
                )

            # ---- main software-pipelined loop ----
            ofs = {}
            for t in range(NTB):
                if t + 1 < NTB:
                    xts[t + 1] = p1x.tile([128, NCT, 512], F32R, tag="xch",
                                          name=f"xch{t + 1}")
                    nc.sync.dma_start(
                        out=xts[t + 1][:],
                        in_=xT_r[:, :, ts(t + 1, 512)].bitcast(F32R),
                    )
                proj(t, xts[t])
                if t == 0:
                    # Wo needed from outproj(0); load + cast after proj(0)
                    wo_f32 = work2.tile([128, NCT, COC], F32, tag="wof")
                    nc.sync.dma_start(
                        out=wo_f32[:],
                        in_=woT_d.rearrange("(a p) t -> p a t", p=128),
                    )
                    nc.vector.tensor_copy(woT_bf[:], wo_f32[:])
                for qc in (2 * t, 2 * t + 1):
                    ofs[qc] = attn(qc)
                    # output projection lags 2 chunks so the AllGather
                    # latency never stalls the PE stream
                    if qc >= 2:
                        outproj(qc - 2, ofs[qc - 2])
            outproj(NQC - 2, ofs[NQC - 2])
            outproj(NQC - 1, ofs[NQC - 1])

    nc.compile()
    return nc


_NC_CACHE = None


def _get_nc():
    global _NC_CACHE
    if _NC_CACHE is None:
        _NC_CACHE = build_nc()
    return _NC_CACHE


def make_in_maps(x, Wq, Wk, Wv, Wo):
    x = np.asarray(x, dtype=np.float32)
    in_maps = []
    for c in range(N_CORES):
        b, g = c // 4, c % 4
        sl = slice(COC * g, COC * g + COC)
        in_maps.append(
            {
                "xT": np.ascontiguousarray(x[b].T),
                "wqT": np.ascontiguousarray(np.asarray(Wq)[sl, :].T),
                "wkT": np.ascontiguousarray(np.asarray(Wk)[sl, :].T),
                "wvT": np.ascontiguousarray(np.asarray(Wv)[sl, :].T),
                "woT": np.ascontiguousarray(np.asarray(Wo)[sl, :].T),
            }
        )
    return in_maps


def assemble(results):
    out = np.empty((B, T, C), dtype=np.float32)
    for c in range(N_CORES):
        b, g = c // 4, c % 4
        out[b, :, COC * g : COC * g + COC] = results[c]["out"].T
    return out


def kernel(x, Wq, Wk, Wv, Wo):
    nc = _get_nc()
    in_maps = make_in_maps(x, Wq, Wk, Wv, Wo)
    res = run_bass_kernel_spmd(nc, in_maps, list(range(N_CORES)))
    return assemble(res.results)


if __name__ == "__main__":
    rng = np.random.default_rng(0)
    x = rng.standard_normal((B, T, C), dtype=np.float32)
    s = 1.0 / np.sqrt(C)
    ws = [
        rng.uniform(-s, s, size=(C, C)).astype(np.float32) for _ in range(4)
    ]
    out = kernel(x, *ws)
    print("kernel ran; out", out.shape, out.dtype)
